# revision 19
# baseline (speedup 1.0000x reference)
"""Multi-head attention on 8 Trainium2 NeuronCores (Bass/Tile).

Problem: B=4, T=2048, DIM=2048, H=16 heads, dk=dv=64.
  q = Q@Wq, k = K@Wk, v = V@Wv  (per head slices)
  out = softmax(q k^T / sqrt(dk)) v @ Wo

Sharding: data-parallel over batch (4) x query-row halves (2) = 8 cores.
Core (b, s) computes output rows [s*1024:(s+1)*1024] of batch b.
Each core projects k/v for the FULL T (recomputing the partner half:
2-rank AllGathers measured ~90us+ fixed latency each on this runtime,
more than the 109us of matmul they would save). Attention + output
projection are core-local; no collectives.

Device layouts (bf16 compute, fp32 PSUM accumulation):
  xqT [D, TQ] / xkT,xvT [D, T] host-transposed (D = contraction dim on
    partitions), wq/wk/wv [D, QK], wo [QK, D] natural (lhsT-ready)
  kT [QK, T]: head h rows 64h..64h+63 -> S^T matmul lhsT
  vaug [T, H, 65]: per head 64 v-cols + ones column (-> softmax row sums)
  Scores per head pair g: S^T chunk [128, 1024] computed as TWO row-tiled
    matmuls (K=dk=64): head 2g in PE row-tile (0,0) -> cols 0:512, head
    2g+1 in row-tile (64,0) -> cols 512:1024. Adjacent instructions on
    disjoint row groups execute concurrently in the PE array.
  P^T = exp(S^T/8)  (scores bounded ~+-5 -> no max-subtraction pass)
  pav [65, TQ-block] per head = vaug.T @ P^T accumulated over key chunks;
    row 64 = denominators l; rows/l via reciprocal_approx_fast +
    DRAM-bounce broadcast of 1/l for both heads at once.
  exp split between ScalarE (12/16 chunks, table exp) and DVE (4/16,
    one fused mul-add with int16 convert bitcast as bf16 = Schraudolph
    approx): the scalar engine alone is the attention bottleneck, and
    the PE idle it causes re-throttles the HAM clock to 1.2 GHz.
  out rows = aoT.T @ Wo accumulated over QK chunks.
"""

import os

import ml_dtypes
import numpy as np

import concourse.bass as bass
from concourse import bacc
import concourse.mybir as mybir
import concourse.tile as tile
from concourse.bass_utils import run_bass_kernel_spmd

BF16 = ml_dtypes.bfloat16
BF = mybir.dt.bfloat16
FP32 = mybir.dt.float32

B = 4
T = 2048
D = 2048
H = 16
DKH = 64
QK = H * DKH   # 1024
TQ = T // 2    # per-core query rows / local T half
KD = D // 128  # 16 contraction chunks for projections
NCORES = 8
EXP_A = 0.125 * 128.0 / float(np.log(2.0))  # score scale folded in
EXP_B = 16250.0  # 127*128 - 6: centers the Schraudolph sawtooth

LAST = None  # BassKernelResults of the most recent run (for test harness)

_cache = {}


def _install_ntff_shim():
    """Provide antenv.axon_hooks + disable artifact upload so that
    run_bass_kernel_spmd(trace=True) can profile under axon in this image."""
    import sys
    import types

    try:
        import antenv.axon_hooks  # noqa: F401
    except ImportError:
        import antenv
        mod = types.ModuleType("antenv.axon_hooks")
        _h = [None]
        mod.set_axon_ntff_profile_hook = lambda h: _h.__setitem__(0, h)
        mod.get_axon_ntff_profile_hook = lambda: _h[0]
        sys.modules["antenv.axon_hooks"] = mod
        antenv.axon_hooks = mod
        try:
            from trn_agent_boot.trn_boot import _ntff_profile_via_ctypes
            mod.set_axon_ntff_profile_hook(
                _ntff_profile_via_ctypes("/opt/axon/libaxon_pjrt.so"))
        except Exception as e:
            print(f"ntff hook registration failed: {e}")
    try:
        import concourse.bass_utils as bu
        bu.upload_artifacts = lambda tmpdir: f"local:{tmpdir}"
    except Exception:
        pass


def _emit(tc, xqT, xkT, xvT, wq, wk, wv, wo, out):
    nc = tc.nc
    exp_f = mybir.ActivationFunctionType.Exp

    with tc.tile_pool(name="persist", bufs=1) as persist:
        kT = persist.tile([128, QK // 128, T], BF, tag="kT")
        vaug = persist.tile([128, T // 128, H, DKH + 1], BF, tag="vaug")
        qT = persist.tile([128, QK // 128, TQ], BF, tag="qT")
        aoT = persist.tile([128, QK // 128, TQ], BF, tag="aoT")
        nc.vector.memset(vaug[:, :, :, DKH:DKH + 1], 1.0)

        # ---- phase 1: k/v projections for the FULL T (recompute partner
        # half: 2-rank collectives measured ~90us+ fixed latency, far more
        # than the 109us of matmul they would save) ----
        with (
            nc.named_scope("p1_kvproj"),
            tc.tile_pool(name="wkv", bufs=1) as wkv_pool,
            tc.tile_pool(name="xk", bufs=17) as xk_pool,
            tc.tile_pool(name="xv", bufs=17) as xv_pool,
            tc.tile_pool(name="ps1", bufs=6, space="PSUM") as ps1,
        ):
            wk_sb = wkv_pool.tile([128, KD, QK], BF, tag="wk")
            wv_sb = wkv_pool.tile([128, KD, QK], BF, tag="wv")

            for nb in range(T // 512):  # 4 blocks, full T
                xk_t = []
                xv_t = []
                for k in range(KD):
                    xkt = xk_pool.tile([128, 512], BF, tag="xk")
                    xvt = xv_pool.tile([128, 512], BF, tag="xv")
                    # interleave weight-chunk and x-chunk loads so the first
                    # matmuls' inputs arrive first
                    if nb == 0:
                        nc.sync.dma_start(out=wk_sb[:, k, :], in_=wk[k * 128:(k + 1) * 128, :])
                        nc.sync.dma_start(out=wv_sb[:, k, :], in_=wv[k * 128:(k + 1) * 128, :])
                    nc.sync.dma_start(out=xkt, in_=xkT[k * 128:(k + 1) * 128, nb * 512:(nb + 1) * 512])
                    nc.sync.dma_start(out=xvt, in_=xvT[k * 128:(k + 1) * 128, nb * 512:(nb + 1) * 512])
                    xk_t.append(xkt)
                    xv_t.append(xvt)
                # kT[m-slice, this T block] = wk_slice.T @ xk
                for m in range(QK // 128):  # 8
                    ps = ps1.tile([128, 512], FP32, tag="ps1")
                    for k in range(KD):
                        nc.tensor.matmul(
                            ps, wk_sb[:, k, m * 128:(m + 1) * 128], xk_t[k],
                            start=(k == 0), stop=(k == KD - 1))
                    nc.vector.tensor_copy(out=kT[:, m, nb * 512:(nb + 1) * 512], in_=ps)
                # v[T-row slice, V cols] = xv_slice.T @ wv
                for msl in range(4):
                    ms = nb * 4 + msl
                    for n in range(QK // 512):  # 2
                        ps = ps1.tile([128, 512], FP32, tag="ps1")
                        for k in range(KD):
                            nc.tensor.matmul(
                                ps, xv_t[k][:, msl * 128:(msl + 1) * 128],
                                wv_sb[:, k, n * 512:(n + 1) * 512],
                                start=(k == 0), stop=(k == KD - 1))
                        nc.vector.tensor_copy(
                            out=vaug[:, ms, n * 8:(n + 1) * 8, 0:DKH],
                            in_=ps.rearrange("p (h d) -> p h d", d=DKH))

                if nb == T // 512 - 1:
                    # keep the PE array busy across the phase transition: a
                    # >3.4us idle gap lets the HAM re-throttle the clock to
                    # 1.2GHz for the next ~30us window. These filler matmuls
                    # read the last x tiles (so they schedule at the phase
                    # tail); their results are never read.
                    for i in range(14):
                        ps = ps1.tile([128, 512], FP32, tag="ps1")
                        nc.tensor.matmul(
                            ps, xv_t[i % KD][:, 0:128], xk_t[(i + 1) % KD],
                            start=True, stop=True)

        # ---- phase 2: q projection + attention ----
        with (
            nc.named_scope("p2_attn"),
            tc.tile_pool(name="wqp", bufs=1) as wq_pool,
            tc.tile_pool(name="xq", bufs=1) as xq_pool,
            tc.tile_pool(name="pt", bufs=13) as pt_pool,
            tc.tile_pool(name="dv", bufs=1) as dv_pool,
            tc.tile_pool(name="dsc", bufs=4, space="DRAM") as dr_pool,
            tc.tile_pool(name="psq", bufs=2, space="PSUM") as ps_q,
            tc.tile_pool(name="pss", bufs=2, space="PSUM") as ps_s,
            tc.tile_pool(name="pav", bufs=2, space="PSUM") as ps_av,
        ):
            wq_sb = wq_pool.tile([128, KD, QK], BF, tag="wq")
            xq_sb = xq_pool.tile([128, KD, TQ], BF, tag="xq")
            for k in range(KD):
                nc.sync.dma_start(out=wq_sb[:, k, :], in_=wq[k * 128:(k + 1) * 128, :])
                nc.sync.dma_start(out=xq_sb[:, k, :], in_=xqT[k * 128:(k + 1) * 128, :])

            # q projection: PE work that runs while the AllGathers fly
            for g in range(QK // 128):  # 8 head pairs
                for n in range(TQ // 512):  # 2
                    ps = ps_q.tile([128, 512], FP32, tag="psq")
                    for k in range(KD):
                        nc.tensor.matmul(
                            ps, wq_sb[:, k, g * 128:(g + 1) * 128],
                            xq_sb[:, k, n * 512:(n + 1) * 512],
                            start=(k == 0), stop=(k == KD - 1))
                    nc.vector.tensor_copy(out=qT[:, g, n * 512:(n + 1) * 512], in_=ps)

            NCH = T // 128  # 16 key chunks
            for g in range(QK // 128):  # 8 head pairs
                for tqb in range(TQ // 512):  # 2
                    qs_a = qT[0:64, g, tqb * 512:(tqb + 1) * 512]
                    qs_b = qT[64:128, g, tqb * 512:(tqb + 1) * 512]
                    pav_a = ps_av.tile([DKH + 1, 512], FP32, tag="pav")
                    pav_b = ps_av.tile([DKH + 1, 512], FP32, tag="pav")
                    pts = [None] * NCH

                    def scores(c):
                        # two row-tiled matmuls per key chunk run
                        # concurrently (head 2g rows 0:64, head 2g+1 rows
                        # 64:128 of the PE array)
                        pss = ps_s.tile([128, 1024], FP32, tag="pss")
                        nc.tensor.matmul(
                            pss[:, 0:512],
                            kT[0:64, g, c * 128:(c + 1) * 128],
                            qs_a, start=True, stop=True)
                        nc.tensor.matmul(
                            pss[:, 512:1024],
                            kT[64:128, g, c * 128:(c + 1) * 128],
                            qs_b, start=True, stop=True)
                        ptt = pt_pool.tile([128, 1024], BF, tag="pt")
                        if c in (2, 5, 7, 10, 13, 15):
                            # Schraudolph exp on DVE: bits(bf16 e^y) ~=
                            # int16(y*128/ln2 + 127*128 - 6): one fused
                            # mul-add with int16 convert, bitcast bf16
                            nc.vector.tensor_scalar(
                                out=ptt.bitcast(mybir.dt.int16), in0=pss,
                                scalar1=EXP_A, scalar2=EXP_B,
                                op0=mybir.AluOpType.mult,
                                op1=mybir.AluOpType.add)
                        else:
                            nc.scalar.activation(out=ptt, in_=pss, func=exp_f, scale=0.125)
                        pts[c] = ptt

                    def av(hp, c):
                        # one PSUM bank per block of 8: alternating the
                        # accumulation target every matmul costs ~125ns each
                        pav, off = (pav_a, 0) if hp == 0 else (pav_b, 512)
                        nc.tensor.matmul(
                            pav, vaug[:, c, 2 * g + hp, :],
                            pts[c][:, off:off + 512],
                            start=(c == 0), stop=(c == NCH - 1))

                    # blocks of 8 so pts 0-7 free before scores 8-15 produce
                    for c in range(0, 8):
                        scores(c)
                    for c in range(0, 8):
                        av(0, c)
                    for c in range(0, 8):
                        av(1, c)
                    for c in range(8, 16):
                        scores(c)
                    for c in range(8, 16):
                        av(0, c)
                    for c in range(8, 16):
                        av(1, c)
                    # Division tail. DVE does only PSUM-reading, DMA-free
                    # work (so the in-order DVE queue never blocks on a DMA
                    # and starves the Schraudolph exps the PE av chains
                    # need); the DRAM-bounce broadcast and the final muls run
                    # on gpsimd, where queue latency is harmless: aoT is not
                    # read until phase 3.
                    att_raw = dv_pool.tile([128, 512], FP32, tag="attr")
                    nc.vector.tensor_copy(out=att_raw[0:DKH, :], in_=pav_a[0:DKH, :])
                    nc.vector.tensor_copy(out=att_raw[DKH:128, :], in_=pav_b[0:DKH, :])
                    lr_a = dv_pool.tile([1, 512], FP32, tag="lra")
                    lr_b = dv_pool.tile([1, 512], FP32, tag="lrb")
                    nc.vector.tensor_copy(out=lr_a, in_=pav_a[DKH:DKH + 1, :])
                    nc.vector.tensor_copy(out=lr_b, in_=pav_b[DKH:DKH + 1, :])
                    linv_a = dv_pool.tile([1, 512], FP32, tag="linva")
                    linv_b = dv_pool.tile([1, 512], FP32, tag="linvb")
                    nc.vector.reciprocal_approx_fast(out=linv_a, in_=lr_a)
                    nc.vector.reciprocal_approx_fast(out=linv_b, in_=lr_b)
                    ldr = dr_pool.tile([2, 512], FP32, tag="ldr")
                    nc.gpsimd.dma_start(out=ldr[0:1, :], in_=linv_a)
                    nc.gpsimd.dma_start(out=ldr[1:2, :], in_=linv_b)
                    lbc = dv_pool.tile([128, 512], FP32, tag="lbc")
                    nc.gpsimd.dma_start(
                        out=lbc[0:DKH, :], in_=ldr[0:1, :].to_broadcast([DKH, 512]))
                    nc.gpsimd.dma_start(
                        out=lbc[DKH:128, :], in_=ldr[1:2, :].to_broadcast([DKH, 512]))
                    att = dv_pool.tile([128, 512], BF, tag="att")
                    # the very last tile goes through DVE: the gpsimd queue
                    # lag would delay aoT[:,7] and stall the start of p3
                    mul_eng = nc.vector if (g == 7 and tqb == 1) else nc.gpsimd
                    mul_eng.tensor_mul(
                        out=att[0:DKH, :], in0=att_raw[0:DKH, :], in1=lbc[0:DKH, :])
                    mul_eng.tensor_mul(
                        out=att[DKH:128, :], in0=att_raw[DKH:128, :], in1=lbc[DKH:128, :])
                    nc.sync.dma_start(
                        out=aoT[:, g, tqb * 512:(tqb + 1) * 512], in_=att)

        # ---- phase 3: output projection ----
        with (
            nc.named_scope("p3_oproj"),
            tc.tile_pool(name="wo", bufs=32) as wo_pool,
            tc.tile_pool(name="ostg", bufs=6) as o_pool,
            tc.tile_pool(name="pso", bufs=6, space="PSUM") as ps_o,
        ):
            KO = QK // 128  # 8
            # all wo tiles upfront: no p2 dependency, so these DMAs land
            # during the attention tail and the first chains never wait
            wo_t = {}
            for nb in range(D // 512):  # 4
                for k in range(KO):
                    wot = wo_pool.tile([128, 512], BF, tag="wo")
                    nc.sync.dma_start(out=wot, in_=wo[k * 128:(k + 1) * 128, nb * 512:(nb + 1) * 512])
                    wo_t[(nb, k)] = wot
            # PE warmth bridge for the p2->p3 transition: reads aoT slice 6
            # (written near the end of attention) so the scheduler places
            # these in the gap before the first real MMs.
            for i in range(12):
                ps = ps_o.tile([128, 512], FP32, tag="pso")
                nc.tensor.matmul(
                    ps, aoT[:, 6, i * 128 % TQ:(i * 128 % TQ) + 128],
                    aoT[:, 6, 0:512], start=True, stop=True)
            for nb in range(D // 512):  # 4
                for m in range(TQ // 128):  # 8
                    ps = ps_o.tile([128, 512], FP32, tag="pso")
                    for k in range(KO):
                        nc.tensor.matmul(
                            ps, aoT[:, k, m * 128:(m + 1) * 128], wo_t[(nb, k)],
                            start=(k == 0), stop=(k == KO - 1))
                    stg = o_pool.tile([128, 512], FP32, tag="ostg")
                    nc.vector.tensor_copy(out=stg, in_=ps)
                    # alternate queues so the final stores drain in parallel
                    eng = nc.sync if (m % 2 == 0) else nc.gpsimd
                    eng.dma_start(
                        out=out[m * 128:(m + 1) * 128, nb * 512:(nb + 1) * 512], in_=stg)


def _build():
    if "nc" in _cache:
        return _cache["nc"]
    nc = bacc.Bacc("TRN2", target_bir_lowering=False, debug=False, num_devices=NCORES)
    xqT = nc.dram_tensor("xqT", [D, TQ], BF, kind="ExternalInput").ap()
    xkT = nc.dram_tensor("xkT", [D, T], BF, kind="ExternalInput").ap()
    xvT = nc.dram_tensor("xvT", [D, T], BF, kind="ExternalInput").ap()
    wq = nc.dram_tensor("wq", [D, QK], BF, kind="ExternalInput").ap()
    wk = nc.dram_tensor("wk", [D, QK], BF, kind="ExternalInput").ap()
    wv = nc.dram_tensor("wv", [D, QK], BF, kind="ExternalInput").ap()
    wo = nc.dram_tensor("wo", [QK, D], BF, kind="ExternalInput").ap()
    out = nc.dram_tensor("out", [TQ, D], mybir.dt.float32, kind="ExternalOutput").ap()
    with tile.TileContext(nc) as tc:
        _emit(tc, xqT, xkT, xvT, wq, wk, wv, wo, out)
    nc.compile()
    _cache["nc"] = nc
    return nc


def kernel(**inputs):
    global LAST
    Q = np.asarray(inputs["Q"], dtype=np.float32)
    K = np.asarray(inputs["K"], dtype=np.float32)
    V = np.asarray(inputs["V"], dtype=np.float32)
    wq_b = np.asarray(inputs["Wq"], dtype=np.float32).astype(BF16)
    wk_b = np.asarray(inputs["Wk"], dtype=np.float32).astype(BF16)
    wv_b = np.asarray(inputs["Wv"], dtype=np.float32).astype(BF16)
    wo_b = np.asarray(inputs["Wo"], dtype=np.float32).astype(BF16)

    nc = _build()
    in_maps = []
    for core in range(NCORES):
        b, s = core // 2, core % 2
        in_maps.append({
            "xqT": np.ascontiguousarray(Q[b, s * TQ:(s + 1) * TQ, :].T).astype(BF16),
            "xkT": np.ascontiguousarray(K[b].T).astype(BF16),
            "xvT": np.ascontiguousarray(V[b].T).astype(BF16),
            "wq": wq_b, "wk": wk_b, "wv": wv_b, "wo": wo_b,
        })
    want_trace = bool(os.environ.get("BASS_TRACE"))
    if want_trace:
        _install_ntff_shim()
        try:
            res = run_bass_kernel_spmd(
                nc, in_maps, core_ids=list(range(NCORES)), trace=True)
        except Exception as e:  # profiling infra missing -> still get results
            print(f"trace run failed ({type(e).__name__}: {e}); retrying untraced")
            res = run_bass_kernel_spmd(nc, in_maps, core_ids=list(range(NCORES)))
    else:
        res = run_bass_kernel_spmd(nc, in_maps, core_ids=list(range(NCORES)))
    LAST = res
    if res.exec_time_ns is not None:
        print(f"HW exec time: {res.exec_time_ns} ns")

    out = np.empty((B, T, D), np.float32)
    for core in range(NCORES):
        b, s = core // 2, core % 2
        out[b, s * TQ:(s + 1) * TQ, :] = res.results[core]["out"]
    return out
